# revision 15
# baseline (speedup 1.0000x reference)
"""Trainium2 Bass kernel: edge-MLP + per-source-node segment softmax / top-k masking.

Graph: N=50000 nodes, DEG=16 edges/node (E=800000), D=128 features.
Sharding: edges (and their source nodes) split into 8 contiguous ranges, one
per NeuronCore (graph/edge parallelism per the sharding hint).

Division of labor (same contract as the original baseline, which already
derived u = f@w1[:D]+b1, v = f@w1[D:2D] and folded w3 into w2 on the host):
the host does the O(E*D) data layout — assembling the layer-1 preactivation
hpre[e] = u[row_e] + v[col_e] + w1c*val_e per edge, int16-quantized (a fixed
+-8 range, ~1.2e-4 abs err) and pre-transposed feature-major per 8192-edge
superblock. All O(E*D^2) MLP FLOPs and the full segment-softmax /
hard-concrete / top-k threshold / masking graph run on device inside
hardware For_i loops (tiny static program, real work in the loop body):

  - DMA hpreT superblock [128, 8192] int16 (sequential, 2 MB)
  - DVE dequant+relu in one op -> h1 (f32)
  - PE layer-2 matmuls (512-wide PSUM banks) with w2*|w3|, ACT relu(+b2*|w3|)
  - PE sign(w3)-sum -> per-edge logit z (softmax-shift-invariant terms dropped)
  - segment softmax -> hard-concrete -> second softmax -> Batcher sort of each
    16-edge group -> 8th-largest threshold -> mask -> scatter to output
"""
import math
import os
from contextlib import ExitStack

import numpy as np

import concourse.bacc as bacc
import concourse.bass as bass
import concourse.tile as tile
from concourse.bass import ds, ts
from concourse import mybir
from concourse.bass_utils import run_bass_kernel_spmd

P = 128
F32 = mybir.dt.float32
I16 = mybir.dt.int16
TT = mybir.AluOpType

N_CORES = 8
N_NODES = 50000
DEG = 16
D = 128
NODES_PC = N_NODES // N_CORES          # 6250
BN = 512                               # nodes per superblock
BE = BN * DEG                          # 8192 edges per superblock
NB = 14                                # padded to even superblock count
QSCALE = 8.0 / 32767.0                 # int16 quant step for hpre
SEG_NB = (NODES_PC + P - 1) // P       # 49 node-columns of 128 in segment stage


def _batcher_substages(n=16):
    t = int(math.ceil(math.log2(n)))
    subs = []
    p = 2 ** (t - 1)
    while p > 0:
        q = 2 ** (t - 1)
        r, d = 0, p
        while True:
            subs.append((p, d, r))
            if q == p:
                break
            d = q - p
            q //= 2
            r = p
        p //= 2
    return subs


def _substage_pattern(n, p, d, r):
    los = [i for i in range(n - d) if (i & p) == r]
    if not los:
        return None
    runs = []
    for i in los:
        if runs and i == runs[-1][0] + runs[-1][1]:
            runs[-1][1] += 1
        else:
            runs.append([i, 1])
    inner = runs[0][1]
    assert all(rn[1] == inner for rn in runs)
    step = runs[1][0] - runs[0][0] if len(runs) > 1 else 1
    assert all(runs[k][0] == runs[0][0] + k * step for k in range(len(runs)))
    return runs[0][0], step, len(runs), inner


EXP_COEF = [np.float64(0.9999999995114079), np.float64(0.9999999374035076), np.float64(0.49999867939792075), np.float64(0.16665600674653547), np.float64(0.04162415464268321), np.float64(0.008240356910079072), np.float64(0.0012740899603045364), np.float64(0.00012118171798647381)]


def _emit_poly_exp(nc, seg, nb, x_in0, sub_ap, out_t, quarter, bcast):
    """out = exp(x_in0 - sub) via degree-7 poly; quarter=True -> 4th-power range trick."""
    r = seg.tile([P, nb, DEG], F32, tag="pe_r", name="pe_r")
    if quarter:
        nc.vector.scalar_tensor_tensor(out=r[:], in0=x_in0, scalar=0.25,
                                       in1=sub_ap, op0=TT.mult, op1=TT.subtract)
    else:
        nc.vector.tensor_tensor(out=r[:], in0=x_in0, in1=sub_ap, op=TT.subtract)
    sacc = seg.tile([P, nb, DEG], F32, tag="pe_s", name="pe_s")
    nc.vector.tensor_scalar_mul(out=sacc[:], in0=r[:], scalar1=float(EXP_COEF[7]))
    for k in range(6, 0, -1):
        nc.vector.scalar_tensor_tensor(out=sacc[:], in0=sacc[:], scalar=float(EXP_COEF[k]),
                                       in1=r[:], op0=TT.add, op1=TT.mult)
    nc.vector.tensor_scalar_add(out=sacc[:], in0=sacc[:], scalar1=float(EXP_COEF[0]))
    if quarter:
        nc.vector.tensor_tensor(out=r[:], in0=sacc[:], in1=sacc[:], op=TT.mult)
        nc.vector.tensor_tensor(out=out_t[:], in0=r[:], in1=r[:], op=TT.mult)
    else:
        nc.vector.tensor_copy(out=out_t[:], in_=sacc[:])


def build_program(inv_temp, repeat=1, debug=False):
    nodes_pc, nb, snb = NODES_PC, NB, SEG_NB
    nc = bacc.Bacc()
    hp_d = nc.declare_dram_parameter("hpT", [nb * P, BE], I16, isOutput=False)
    # packed consts: [:,0:128]=w2p, [:,128]=sgn, [:,129]=b2p
    cst_d = nc.declare_dram_parameter("cst", [P, 130], F32, isOutput=False)
    out_d = nc.declare_dram_parameter("out", [nodes_pc * DEG], F32, isOutput=True)
    zscr_d = nc.dram_tensor("z_scratch", [nb * BE], F32)

    with tile.TileContext(nc) as tc, ExitStack() as ctx:
        const = ctx.enter_context(tc.tile_pool(name="const", bufs=1))
        cst_sb = const.tile([P, 130], F32)
        nc.sync.dma_start(out=cst_sb[:], in_=cst_d[:])
        w2p_sb = cst_sb[:, 0:128]
        sgn_sb = cst_sb[:, 128:129]
        b2p_sb = cst_sb[:, 129:130]
        eps_sb = const.tile([P, 1], F32)
        nc.vector.memset(eps_sb[:], 1e-8)
        zn_sb = const.tile([P, snb, DEG], F32)

        hp_pool = ctx.enter_context(tc.tile_pool(name="hpp", bufs=2))
        h1_pool = ctx.enter_context(tc.tile_pool(name="h1p", bufs=1))
        s_pool = ctx.enter_context(tc.tile_pool(name="sp", bufs=1))
        z_pool = ctx.enter_context(tc.tile_pool(name="zp", bufs=1))
        ps_s = ctx.enter_context(tc.tile_pool(name="pss", bufs=1, space="PSUM"))
        ps_z = ctx.enter_context(tc.tile_pool(name="psz", bufs=1, space="PSUM"))

        rep_ctx = tc.For_i(0, repeat, name="rep")
        _rep = rep_ctx.__enter__()
        blk_ctx = tc.For_i(0, nb, step=2, name="blk")
        bi = blk_ctx.__enter__()
        for u in range(2):
            hp_sb = hp_pool.tile([P, BE], I16, tag=f"hp{u}", name=f"hp{u}")
            nc.sync.dma_start(out=hp_sb[:], in_=hp_d[ts(bi + u, P)])
            h1r = h1_pool.tile([P, BE], F32, tag=f"h1{u}", name=f"h1{u}")
            # dequant + relu in one op
            nc.vector.tensor_scalar(out=h1r[:], in0=hp_sb[:], scalar1=QSCALE,
                                    scalar2=0.0, op0=TT.mult, op1=TT.max)
            z_sb = z_pool.tile([1, BE], F32, tag="z", name="z_sb")
            for g in range(BE // 2048):
                pss = ps_s.tile([P, 2048], F32, tag="pss", name="pss")
                for c in range(4):
                    nc.tensor.matmul(out=pss[:, 512 * c:512 * (c + 1)],
                                     lhsT=w2p_sb,
                                     rhs=h1r[:, 2048 * g + 512 * c:2048 * g + 512 * (c + 1)],
                                     start=True, stop=True)
                s_sb = s_pool.tile([P, 2048], F32, tag=f"s{u}", name=f"s{u}")
                nc.scalar.activation(out=s_sb[:], in_=pss[:],
                                     func=mybir.ActivationFunctionType.Relu,
                                     bias=b2p_sb)
                psz = ps_z.tile([1, 2048], F32, tag="psz", name="psz")
                for c in range(4):
                    nc.tensor.matmul(out=psz[:, 512 * c:512 * (c + 1)],
                                     lhsT=sgn_sb, rhs=s_sb[:, 512 * c:512 * (c + 1)],
                                     start=True, stop=True)
                zc = z_sb[:, 2048 * g:2048 * (g + 1)]
                if g % 2 == 0:
                    nc.vector.tensor_copy(out=zc, in_=psz[:])
                else:
                    nc.scalar.activation(out=zc, in_=psz[:],
                                         func=mybir.ActivationFunctionType.Copy)
            nc.sync.dma_start(out=zscr_d[ts(bi + u, BE)], in_=z_sb[:])
        blk_ctx.__exit__(None, None, None)

        # segment stage: fetch z back node-major [p, b, j] <- z_scratch[b*2048 + p*16 + j]
        znsrc = bass.AP(tensor=zscr_d[:].tensor, offset=zscr_d[:].offset,
                        ap=[[snb * DEG, P], [DEG, snb], [1, DEG]])
        nc.sync.dma_start(out=zn_sb[:], in_=znsrc)
        seg = ctx.enter_context(tc.tile_pool(name="seg", bufs=1))

        def t_new(nm):
            t = seg.tile([P, snb, DEG], F32, tag=nm, name=nm)
            return t

        def bcast(t2):
            a = t2[:]
            return bass.AP(tensor=a.tensor, offset=a.offset,
                           ap=[list(a.ap[0]), list(a.ap[1]), [0, DEG]])

        m1 = seg.tile([P, snb], F32)
        nc.vector.reduce_max(out=m1[:], in_=zn_sb[:], axis=mybir.AxisListType.X)
        e1 = t_new("e1")
        nc.vector.tensor_tensor(out=e1[:], in0=zn_sb[:], in1=bcast(m1),
                                op=TT.subtract)
        nc.scalar.activation(out=e1[:], in_=e1[:],
                             func=mybir.ActivationFunctionType.Exp)
        s1 = seg.tile([P, snb], F32)
        nc.vector.reduce_sum(out=s1[:], in_=e1[:], axis=mybir.AxisListType.X)
        r1 = seg.tile([P, snb], F32)
        nc.vector.reciprocal(out=r1[:], in_=s1[:])
        pi = t_new("pi")
        nc.vector.tensor_tensor(out=pi[:], in0=e1[:], in1=bcast(r1), op=TT.mult)
        hard = t_new("hard")
        nc.scalar.activation(out=hard[:], in_=pi[:],
                             func=mybir.ActivationFunctionType.Ln, bias=eps_sb[:])
        nc.scalar.activation(out=hard[:], in_=hard[:],
                             func=mybir.ActivationFunctionType.Sigmoid,
                             scale=float(inv_temp))
        m2 = seg.tile([P, snb], F32)
        nc.vector.reduce_max(out=m2[:], in_=hard[:], axis=mybir.AxisListType.X)
        e2 = t_new("e2")
        nc.vector.tensor_tensor(out=e2[:], in0=hard[:], in1=bcast(m2),
                                op=TT.subtract)
        nc.scalar.activation(out=e2[:], in_=e2[:],
                             func=mybir.ActivationFunctionType.Exp)
        s2 = seg.tile([P, snb], F32)
        nc.vector.reduce_sum(out=s2[:], in_=e2[:], axis=mybir.AxisListType.X)
        r2 = seg.tile([P, snb], F32)
        nc.vector.reciprocal(out=r2[:], in_=s2[:])
        y = t_new("y")
        nc.vector.tensor_tensor(out=y[:], in0=e2[:], in1=bcast(r2), op=TT.mult)

        A = t_new("A")
        nc.vector.tensor_copy(out=A[:], in_=y[:])
        Tt = seg.tile([P, snb, 8], F32)
        for (p_, d_, r_) in _batcher_substages(16):
            pat = _substage_pattern(16, p_, d_, r_)
            if pat is None:
                continue
            off, ostep, ocnt, icnt = pat

            def sl(extra):
                a = A[:]
                return bass.AP(tensor=a.tensor, offset=a.offset + off + extra,
                               ap=[list(a.ap[0]), [DEG, snb], [ostep, ocnt], [1, icnt]])

            tlo, thi = sl(0), sl(d_)
            tt_ap = bass.AP(tensor=Tt[:].tensor, offset=Tt[:].offset,
                            ap=[list(Tt[:].ap[0]), [8, snb], [icnt, ocnt], [1, icnt]])
            nc.vector.tensor_tensor(out=tt_ap, in0=tlo, in1=thi, op=TT.min)
            nc.vector.tensor_tensor(out=tlo, in0=tlo, in1=thi, op=TT.max)
            nc.vector.tensor_copy(out=thi, in_=tt_ap)
        thre = seg.tile([P, snb], F32)
        nc.vector.tensor_copy(out=thre[:], in_=A[:, :, 7])
        g = t_new("g")
        nc.vector.scalar_tensor_tensor(out=g[:], in0=y[:], scalar=1e-7,
                                       in1=bcast(thre), op0=TT.add, op1=TT.is_gt)
        masked = t_new("masked")
        nc.vector.tensor_tensor(out=masked[:], in0=g[:], in1=y[:], op=TT.mult)

        out_main = bass.AP(tensor=out_d[:].tensor, offset=out_d[:].offset,
                           ap=[[snb * DEG, P - 1], [DEG, snb], [1, DEG]])
        nc.sync.dma_start(out=out_main, in_=masked[0:P - 1, :, :])
        tail_cols = nodes_pc - (P - 1) * snb    # 27
        out_last = bass.AP(tensor=out_d[:].tensor,
                           offset=out_d[:].offset + (P - 1) * snb * DEG,
                           ap=[[DEG, tail_cols], [1, DEG]])
        nc.sync.dma_start(out=out_last, in_=masked[P - 1:P, 0:tail_cols, :])
        rep_ctx.__exit__(None, None, None)
    nc.compile()
    return nc


def _host_prepare(features, indices, values, temperature, w1, b1, w2, b2, w3, b3):
    features = np.asarray(features, np.float32)
    row = np.asarray(indices)[0].astype(np.int64)
    col = np.asarray(indices)[1].astype(np.int64)
    val = np.asarray(values, np.float32).reshape(-1)
    w1 = np.asarray(w1, np.float32)
    u_full = (features @ w1[:D] + np.asarray(b1, np.float32)).astype(np.float32)
    v_full = (features @ w1[D:2 * D]).astype(np.float32)
    w1c = w1[2 * D]
    w3c = np.asarray(w3, np.float32)[:, 0]
    aw3 = np.abs(w3c)
    w2p = np.ascontiguousarray((np.asarray(w2, np.float32) * aw3[None, :]).astype(np.float32))
    sgn = np.sign(w3c).astype(np.float32)
    b2p = (np.asarray(b2, np.float32) * aw3).astype(np.float32)
    cst = np.zeros((P, 130), np.float32)
    cst[:, 0:128] = w2p
    cst[:, 128] = sgn
    cst[:, 129] = b2p
    # layer-1 preactivation per edge, int16-quantized feature-major superblocks
    hpre = u_full[row] + v_full[col] + val[:, None] * w1c[None, :]
    hq_all = np.clip(np.round(hpre * (1.0 / QSCALE)), -32767, 32767).astype(np.int16)
    in_maps = []
    for c in range(N_CORES):
        n0 = c * NODES_PC
        e0 = n0 * DEG
        hpT = np.zeros((NB * P, BE), np.int16)
        for b in range(NB):
            bb = b * BN
            n_nodes = max(0, min(BN, NODES_PC - bb))
            if n_nodes == 0:
                continue
            w = n_nodes * DEG
            sl = slice(e0 + bb * DEG, e0 + bb * DEG + w)
            hpT[b * P:(b + 1) * P, 0:w] = hq_all[sl].T
        in_maps.append({"hpT": hpT, "cst": cst})
    return in_maps


_PROGRAM_CACHE = {}


def kernel(features, indices, values, temperature, w1, b1, w2, b2, w3, b3):
    inv_temp = 1.0 / float(np.asarray(temperature))
    in_maps = _host_prepare(features, indices, values, temperature,
                            w1, b1, w2, b2, w3, b3)
    key = ("v9", inv_temp)
    if key not in _PROGRAM_CACHE:
        _PROGRAM_CACHE[key] = build_program(inv_temp)
    nc = _PROGRAM_CACHE[key]
    trace = bool(os.environ.get("TOPK_TRACE"))
    res = run_bass_kernel_spmd(nc, in_maps, list(range(N_CORES)), trace=trace)
    if trace:
        kernel.last_result = res
    out = np.concatenate([res.results[c]["out"] for c in range(N_CORES)])
    return out.astype(np.float32)


# revision 16
# speedup vs baseline: 1.6544x; 1.6544x over previous
"""Trainium2 Bass kernel: edge-MLP + per-source-node segment softmax / top-k masking.

Graph: N=50000 nodes, DEG=16 edges/node (E=800000), D=128 features.
Sharding: edges (and their source nodes) split into 8 contiguous ranges, one
per NeuronCore (graph/edge parallelism per the sharding hint).

Division of labor (same contract as the original baseline, which already
derived u = f@w1[:D]+b1, v = f@w1[D:2D] and folded w3 into w2 on the host):
the host does the O(E*D) data layout — assembling the layer-1 preactivation
hpre[e] = u[row_e] + v[col_e] + w1c*val_e per edge, int16-quantized (a fixed
+-8 range, ~1.2e-4 abs err) and pre-transposed feature-major per 8192-edge
superblock. All O(E*D^2) MLP FLOPs and the full segment-softmax /
hard-concrete / top-k threshold / masking graph run on device inside
hardware For_i loops (tiny static program, real work in the loop body):

  - DMA hpreT superblock [128, 8192] int16 (sequential, 2 MB)
  - DVE dequant+relu in one op -> h1 (f32)
  - PE layer-2 matmuls (512-wide PSUM banks) with w2*|w3|, ACT relu(+b2*|w3|)
  - PE sign(w3)-sum -> per-edge logit z (softmax-shift-invariant terms dropped)
  - segment softmax -> hard-concrete -> second softmax -> Batcher sort of each
    16-edge group -> 8th-largest threshold -> mask -> scatter to output
"""
import math
import os
from contextlib import ExitStack

import numpy as np

import concourse.bacc as bacc
import concourse.bass as bass
import concourse.tile as tile
from concourse.bass import ds, ts
from concourse import mybir
from concourse.bass_utils import run_bass_kernel_spmd

P = 128
F32 = mybir.dt.float32
I16 = mybir.dt.int16
TT = mybir.AluOpType

N_CORES = 8
N_NODES = 50000
DEG = 16
D = 128
NODES_PC = N_NODES // N_CORES          # 6250
BN = 512                               # nodes per superblock
BE = BN * DEG                          # 8192 edges per superblock
NB = 14                                # padded to even superblock count
QSCALE = 8.0 / 32767.0                 # int16 quant step for hpre
SEG_NB = (NODES_PC + P - 1) // P       # 49 node-columns of 128 in segment stage


def _batcher_substages(n=16):
    t = int(math.ceil(math.log2(n)))
    subs = []
    p = 2 ** (t - 1)
    while p > 0:
        q = 2 ** (t - 1)
        r, d = 0, p
        while True:
            subs.append((p, d, r))
            if q == p:
                break
            d = q - p
            q //= 2
            r = p
        p //= 2
    return subs


def _substage_pattern(n, p, d, r):
    los = [i for i in range(n - d) if (i & p) == r]
    if not los:
        return None
    runs = []
    for i in los:
        if runs and i == runs[-1][0] + runs[-1][1]:
            runs[-1][1] += 1
        else:
            runs.append([i, 1])
    inner = runs[0][1]
    assert all(rn[1] == inner for rn in runs)
    step = runs[1][0] - runs[0][0] if len(runs) > 1 else 1
    assert all(runs[k][0] == runs[0][0] + k * step for k in range(len(runs)))
    return runs[0][0], step, len(runs), inner


EXP_COEF = [np.float64(0.9999999995114079), np.float64(0.9999999374035076), np.float64(0.49999867939792075), np.float64(0.16665600674653547), np.float64(0.04162415464268321), np.float64(0.008240356910079072), np.float64(0.0012740899603045364), np.float64(0.00012118171798647381)]


def _emit_poly_exp(nc, seg, nb, x_in0, sub_ap, out_t, quarter, bcast):
    """out = exp(x_in0 - sub) via degree-7 poly; quarter=True -> 4th-power range trick."""
    r = seg.tile([P, nb, DEG], F32, tag="pe_r", name="pe_r")
    if quarter:
        nc.vector.scalar_tensor_tensor(out=r[:], in0=x_in0, scalar=0.25,
                                       in1=sub_ap, op0=TT.mult, op1=TT.subtract)
    else:
        nc.vector.tensor_tensor(out=r[:], in0=x_in0, in1=sub_ap, op=TT.subtract)
    sacc = seg.tile([P, nb, DEG], F32, tag="pe_s", name="pe_s")
    nc.vector.tensor_scalar_mul(out=sacc[:], in0=r[:], scalar1=float(EXP_COEF[7]))
    for k in range(6, 0, -1):
        nc.vector.scalar_tensor_tensor(out=sacc[:], in0=sacc[:], scalar=float(EXP_COEF[k]),
                                       in1=r[:], op0=TT.add, op1=TT.mult)
    nc.vector.tensor_scalar_add(out=sacc[:], in0=sacc[:], scalar1=float(EXP_COEF[0]))
    if quarter:
        nc.vector.tensor_tensor(out=r[:], in0=sacc[:], in1=sacc[:], op=TT.mult)
        nc.vector.tensor_tensor(out=out_t[:], in0=r[:], in1=r[:], op=TT.mult)
    else:
        nc.vector.tensor_copy(out=out_t[:], in_=sacc[:])


def build_program(inv_temp, repeat=1, debug=False):
    nodes_pc, nb, snb = NODES_PC, NB, SEG_NB
    nc = bacc.Bacc()
    hp_d = nc.declare_dram_parameter("hpT", [nb * P, BE], I16, isOutput=False)
    # packed consts: [:,0:128]=w2p, [:,128]=sgn, [:,129]=b2p
    cst_d = nc.declare_dram_parameter("cst", [P, 130], F32, isOutput=False)
    out_d = nc.declare_dram_parameter("out", [nodes_pc * DEG], F32, isOutput=True)
    zscr_d = nc.dram_tensor("z_scratch", [nb * BE], F32)

    with tile.TileContext(nc) as tc, ExitStack() as ctx:
        const = ctx.enter_context(tc.tile_pool(name="const", bufs=1))
        cst_sb = const.tile([P, 130], F32)
        nc.sync.dma_start(out=cst_sb[:], in_=cst_d[:])
        w2p_sb = cst_sb[:, 0:128]
        sgn_sb = cst_sb[:, 128:129]
        b2p_sb = cst_sb[:, 129:130]
        eps_sb = const.tile([P, 1], F32)
        nc.vector.memset(eps_sb[:], 1e-8)
        zn_sb = const.tile([P, snb, DEG], F32)

        hp_pool = ctx.enter_context(tc.tile_pool(name="hpp", bufs=2))
        h1_pool = ctx.enter_context(tc.tile_pool(name="h1p", bufs=1))
        s_pool = ctx.enter_context(tc.tile_pool(name="sp", bufs=1))
        z_pool = ctx.enter_context(tc.tile_pool(name="zp", bufs=1))
        ps_s = ctx.enter_context(tc.tile_pool(name="pss", bufs=1, space="PSUM"))
        ps_z = ctx.enter_context(tc.tile_pool(name="psz", bufs=1, space="PSUM"))

        rep_ctx = tc.For_i(0, repeat, name="rep")
        _rep = rep_ctx.__enter__()
        blk_ctx = tc.For_i(0, nb, step=2, name="blk")
        bi = blk_ctx.__enter__()
        for u in range(2):
            hp_sb = hp_pool.tile([P, BE], I16, tag=f"hp{u}", name=f"hp{u}")
            nc.sync.dma_start(out=hp_sb[:], in_=hp_d[ts(bi + u, P)])
            h1r = h1_pool.tile([P, BE], F32, tag=f"h1{u}", name=f"h1{u}")
            # dequant + relu in one op
            nc.vector.tensor_scalar(out=h1r[:], in0=hp_sb[:], scalar1=QSCALE,
                                    scalar2=0.0, op0=TT.mult, op1=TT.max)
            z_sb = z_pool.tile([1, BE], F32, tag="z", name="z_sb")
            for g in range(BE // 1024):
                k = g % 2
                pss = ps_s.tile([P, 1024], F32, tag=f"pss{k}", name=f"pss{k}")
                for c in range(2):
                    nc.tensor.matmul(out=pss[:, 512 * c:512 * (c + 1)],
                                     lhsT=w2p_sb,
                                     rhs=h1r[:, 1024 * g + 512 * c:1024 * g + 512 * (c + 1)],
                                     start=True, stop=True)
                s_sb = s_pool.tile([P, 1024], F32, tag=f"s{k}", name=f"s{k}")
                nc.scalar.activation(out=s_sb[:], in_=pss[:],
                                     func=mybir.ActivationFunctionType.Relu,
                                     bias=b2p_sb)
                psz = ps_z.tile([1, 1024], F32, tag=f"psz{k}", name=f"psz{k}")
                for c in range(2):
                    nc.tensor.matmul(out=psz[:, 512 * c:512 * (c + 1)],
                                     lhsT=sgn_sb, rhs=s_sb[:, 512 * c:512 * (c + 1)],
                                     start=True, stop=True)
                zc = z_sb[:, 1024 * g:1024 * (g + 1)]
                if k == 0:
                    nc.vector.tensor_copy(out=zc, in_=psz[:])
                else:
                    nc.scalar.activation(out=zc, in_=psz[:],
                                         func=mybir.ActivationFunctionType.Copy)
            nc.sync.dma_start(out=zscr_d[ts(bi + u, BE)], in_=z_sb[:])
        blk_ctx.__exit__(None, None, None)

        # segment stage: fetch z back node-major [p, b, j] <- z_scratch[b*2048 + p*16 + j]
        znsrc = bass.AP(tensor=zscr_d[:].tensor, offset=zscr_d[:].offset,
                        ap=[[snb * DEG, P], [DEG, snb], [1, DEG]])
        nc.sync.dma_start(out=zn_sb[:], in_=znsrc)
        seg = ctx.enter_context(tc.tile_pool(name="seg", bufs=1))

        def t_new(nm):
            t = seg.tile([P, snb, DEG], F32, tag=nm, name=nm)
            return t

        def bcast(t2):
            a = t2[:]
            return bass.AP(tensor=a.tensor, offset=a.offset,
                           ap=[list(a.ap[0]), list(a.ap[1]), [0, DEG]])

        m1 = seg.tile([P, snb], F32)
        nc.vector.reduce_max(out=m1[:], in_=zn_sb[:], axis=mybir.AxisListType.X)
        e1 = t_new("e1")
        nc.vector.tensor_tensor(out=e1[:], in0=zn_sb[:], in1=bcast(m1),
                                op=TT.subtract)
        nc.scalar.activation(out=e1[:], in_=e1[:],
                             func=mybir.ActivationFunctionType.Exp)
        s1 = seg.tile([P, snb], F32)
        nc.vector.reduce_sum(out=s1[:], in_=e1[:], axis=mybir.AxisListType.X)
        r1 = seg.tile([P, snb], F32)
        nc.vector.reciprocal(out=r1[:], in_=s1[:])
        pi = t_new("pi")
        nc.vector.tensor_tensor(out=pi[:], in0=e1[:], in1=bcast(r1), op=TT.mult)
        hard = t_new("hard")
        nc.scalar.activation(out=hard[:], in_=pi[:],
                             func=mybir.ActivationFunctionType.Ln, bias=eps_sb[:])
        nc.scalar.activation(out=hard[:], in_=hard[:],
                             func=mybir.ActivationFunctionType.Sigmoid,
                             scale=float(inv_temp))
        m2 = seg.tile([P, snb], F32)
        nc.vector.reduce_max(out=m2[:], in_=hard[:], axis=mybir.AxisListType.X)
        e2 = t_new("e2")
        nc.vector.tensor_tensor(out=e2[:], in0=hard[:], in1=bcast(m2),
                                op=TT.subtract)
        nc.scalar.activation(out=e2[:], in_=e2[:],
                             func=mybir.ActivationFunctionType.Exp)
        s2 = seg.tile([P, snb], F32)
        nc.vector.reduce_sum(out=s2[:], in_=e2[:], axis=mybir.AxisListType.X)
        r2 = seg.tile([P, snb], F32)
        nc.vector.reciprocal(out=r2[:], in_=s2[:])
        y = t_new("y")
        nc.vector.tensor_tensor(out=y[:], in0=e2[:], in1=bcast(r2), op=TT.mult)

        A = t_new("A")
        nc.vector.tensor_copy(out=A[:], in_=y[:])
        Tt = seg.tile([P, snb, 8], F32)
        for (p_, d_, r_) in _batcher_substages(16):
            pat = _substage_pattern(16, p_, d_, r_)
            if pat is None:
                continue
            off, ostep, ocnt, icnt = pat

            def sl(extra):
                a = A[:]
                return bass.AP(tensor=a.tensor, offset=a.offset + off + extra,
                               ap=[list(a.ap[0]), [DEG, snb], [ostep, ocnt], [1, icnt]])

            tlo, thi = sl(0), sl(d_)
            tt_ap = bass.AP(tensor=Tt[:].tensor, offset=Tt[:].offset,
                            ap=[list(Tt[:].ap[0]), [8, snb], [icnt, ocnt], [1, icnt]])
            nc.vector.tensor_tensor(out=tt_ap, in0=tlo, in1=thi, op=TT.min)
            nc.vector.tensor_tensor(out=tlo, in0=tlo, in1=thi, op=TT.max)
            nc.vector.tensor_copy(out=thi, in_=tt_ap)
        thre = seg.tile([P, snb], F32)
        nc.vector.tensor_copy(out=thre[:], in_=A[:, :, 7])
        g = t_new("g")
        nc.vector.scalar_tensor_tensor(out=g[:], in0=y[:], scalar=1e-7,
                                       in1=bcast(thre), op0=TT.add, op1=TT.is_gt)
        masked = t_new("masked")
        nc.vector.tensor_tensor(out=masked[:], in0=g[:], in1=y[:], op=TT.mult)

        out_main = bass.AP(tensor=out_d[:].tensor, offset=out_d[:].offset,
                           ap=[[snb * DEG, P - 1], [DEG, snb], [1, DEG]])
        nc.sync.dma_start(out=out_main, in_=masked[0:P - 1, :, :])
        tail_cols = nodes_pc - (P - 1) * snb    # 27
        out_last = bass.AP(tensor=out_d[:].tensor,
                           offset=out_d[:].offset + (P - 1) * snb * DEG,
                           ap=[[DEG, tail_cols], [1, DEG]])
        nc.sync.dma_start(out=out_last, in_=masked[P - 1:P, 0:tail_cols, :])
        rep_ctx.__exit__(None, None, None)
    nc.compile()
    return nc


def _host_prepare(features, indices, values, temperature, w1, b1, w2, b2, w3, b3):
    features = np.asarray(features, np.float32)
    row = np.asarray(indices)[0].astype(np.int64)
    col = np.asarray(indices)[1].astype(np.int64)
    val = np.asarray(values, np.float32).reshape(-1)
    w1 = np.asarray(w1, np.float32)
    u_full = (features @ w1[:D] + np.asarray(b1, np.float32)).astype(np.float32)
    v_full = (features @ w1[D:2 * D]).astype(np.float32)
    w1c = w1[2 * D]
    w3c = np.asarray(w3, np.float32)[:, 0]
    aw3 = np.abs(w3c)
    w2p = np.ascontiguousarray((np.asarray(w2, np.float32) * aw3[None, :]).astype(np.float32))
    sgn = np.sign(w3c).astype(np.float32)
    b2p = (np.asarray(b2, np.float32) * aw3).astype(np.float32)
    cst = np.zeros((P, 130), np.float32)
    cst[:, 0:128] = w2p
    cst[:, 128] = sgn
    cst[:, 129] = b2p
    # layer-1 preactivation per edge, int16-quantized feature-major superblocks
    hpre = u_full[row] + v_full[col] + val[:, None] * w1c[None, :]
    hq_all = np.clip(np.round(hpre * (1.0 / QSCALE)), -32767, 32767).astype(np.int16)
    in_maps = []
    for c in range(N_CORES):
        n0 = c * NODES_PC
        e0 = n0 * DEG
        hpT = np.zeros((NB * P, BE), np.int16)
        for b in range(NB):
            bb = b * BN
            n_nodes = max(0, min(BN, NODES_PC - bb))
            if n_nodes == 0:
                continue
            w = n_nodes * DEG
            sl = slice(e0 + bb * DEG, e0 + bb * DEG + w)
            hpT[b * P:(b + 1) * P, 0:w] = hq_all[sl].T
        in_maps.append({"hpT": hpT, "cst": cst})
    return in_maps


_PROGRAM_CACHE = {}


def kernel(features, indices, values, temperature, w1, b1, w2, b2, w3, b3):
    inv_temp = 1.0 / float(np.asarray(temperature))
    in_maps = _host_prepare(features, indices, values, temperature,
                            w1, b1, w2, b2, w3, b3)
    key = ("v10", inv_temp)
    if key not in _PROGRAM_CACHE:
        _PROGRAM_CACHE[key] = build_program(inv_temp)
    nc = _PROGRAM_CACHE[key]
    trace = bool(os.environ.get("TOPK_TRACE"))
    res = run_bass_kernel_spmd(nc, in_maps, list(range(N_CORES)), trace=trace)
    if trace:
        kernel.last_result = res
    out = np.concatenate([res.results[c]["out"] for c in range(N_CORES)])
    return out.astype(np.float32)


# revision 17
# speedup vs baseline: 1.7540x; 1.0602x over previous
"""Trainium2 Bass kernel: edge-MLP + per-source-node segment softmax / top-k masking.

Graph: N=50000 nodes, DEG=16 edges/node (E=800000), D=128 features.
Sharding: edges (and their source nodes) split into 8 contiguous ranges, one
per NeuronCore (graph/edge parallelism per the sharding hint).

Division of labor (same contract as the original baseline, which already
derived u = f@w1[:D]+b1, v = f@w1[D:2D] and folded w3 into w2 on the host):
the host does the O(E*D) data layout — assembling the layer-1 preactivation
hpre[e] = u[row_e] + v[col_e] + w1c*val_e per edge, int16-quantized (a fixed
+-8 range, ~1.2e-4 abs err) and pre-transposed feature-major per 8192-edge
superblock. All O(E*D^2) MLP FLOPs and the full segment-softmax /
hard-concrete / top-k threshold / masking graph run on device inside
hardware For_i loops (tiny static program, real work in the loop body):

  - DMA hpreT superblock [128, 8192] int16 (sequential, 2 MB)
  - DVE dequant+relu in one op -> h1 (f32)
  - PE layer-2 matmuls (512-wide PSUM banks) with w2*|w3|, ACT relu(+b2*|w3|)
  - PE sign(w3)-sum -> per-edge logit z (softmax-shift-invariant terms dropped)
  - segment softmax -> hard-concrete -> second softmax -> Batcher sort of each
    16-edge group -> 8th-largest threshold -> mask -> scatter to output
"""
import math
import os
from contextlib import ExitStack

import numpy as np

import concourse.bacc as bacc
import concourse.bass as bass
import concourse.tile as tile
from concourse.bass import ds, ts
from concourse import mybir
from concourse.bass_utils import run_bass_kernel_spmd

P = 128
F32 = mybir.dt.float32
I16 = mybir.dt.int16
TT = mybir.AluOpType

N_CORES = 8
N_NODES = 50000
DEG = 16
D = 128
NODES_PC = N_NODES // N_CORES          # 6250
BN = 448                               # nodes per superblock
BE = BN * DEG                          # 7168 edges per superblock
NB = 14                                # padded to even superblock count
QSCALE = 8.0 / 32767.0                 # int16 quant step for hpre
SEG_NB = (NODES_PC + P - 1) // P       # 49 node-columns of 128 in segment stage


def _batcher_substages(n=16):
    t = int(math.ceil(math.log2(n)))
    subs = []
    p = 2 ** (t - 1)
    while p > 0:
        q = 2 ** (t - 1)
        r, d = 0, p
        while True:
            subs.append((p, d, r))
            if q == p:
                break
            d = q - p
            q //= 2
            r = p
        p //= 2
    return subs


def _substage_pattern(n, p, d, r):
    los = [i for i in range(n - d) if (i & p) == r]
    if not los:
        return None
    runs = []
    for i in los:
        if runs and i == runs[-1][0] + runs[-1][1]:
            runs[-1][1] += 1
        else:
            runs.append([i, 1])
    inner = runs[0][1]
    assert all(rn[1] == inner for rn in runs)
    step = runs[1][0] - runs[0][0] if len(runs) > 1 else 1
    assert all(runs[k][0] == runs[0][0] + k * step for k in range(len(runs)))
    return runs[0][0], step, len(runs), inner


EXP_COEF = [np.float64(0.9999999995114079), np.float64(0.9999999374035076), np.float64(0.49999867939792075), np.float64(0.16665600674653547), np.float64(0.04162415464268321), np.float64(0.008240356910079072), np.float64(0.0012740899603045364), np.float64(0.00012118171798647381)]


def _emit_poly_exp(nc, seg, nb, x_in0, sub_ap, out_t, quarter, bcast):
    """out = exp(x_in0 - sub) via degree-7 poly; quarter=True -> 4th-power range trick."""
    r = seg.tile([P, nb, DEG], F32, tag="pe_r", name="pe_r")
    if quarter:
        nc.vector.scalar_tensor_tensor(out=r[:], in0=x_in0, scalar=0.25,
                                       in1=sub_ap, op0=TT.mult, op1=TT.subtract)
    else:
        nc.vector.tensor_tensor(out=r[:], in0=x_in0, in1=sub_ap, op=TT.subtract)
    sacc = seg.tile([P, nb, DEG], F32, tag="pe_s", name="pe_s")
    nc.vector.tensor_scalar_mul(out=sacc[:], in0=r[:], scalar1=float(EXP_COEF[7]))
    for k in range(6, 0, -1):
        nc.vector.scalar_tensor_tensor(out=sacc[:], in0=sacc[:], scalar=float(EXP_COEF[k]),
                                       in1=r[:], op0=TT.add, op1=TT.mult)
    nc.vector.tensor_scalar_add(out=sacc[:], in0=sacc[:], scalar1=float(EXP_COEF[0]))
    if quarter:
        nc.vector.tensor_tensor(out=r[:], in0=sacc[:], in1=sacc[:], op=TT.mult)
        nc.vector.tensor_tensor(out=out_t[:], in0=r[:], in1=r[:], op=TT.mult)
    else:
        nc.vector.tensor_copy(out=out_t[:], in_=sacc[:])


def build_program(inv_temp, repeat=1, debug=False):
    nodes_pc, nb, snb = NODES_PC, NB, SEG_NB
    nc = bacc.Bacc()
    hp_d = nc.declare_dram_parameter("hpT", [nb * P, BE], I16, isOutput=False)
    # packed consts: [:,0:128]=w2p, [:,128]=sgn, [:,129]=b2p
    cst_d = nc.declare_dram_parameter("cst", [P, 130], F32, isOutput=False)
    out_d = nc.declare_dram_parameter("out", [nodes_pc * DEG], F32, isOutput=True)
    zscr_d = nc.dram_tensor("z_scratch", [nb * BE], F32)

    with tile.TileContext(nc) as tc, ExitStack() as ctx:
        const = ctx.enter_context(tc.tile_pool(name="const", bufs=1))
        cst_sb = const.tile([P, 130], F32)
        nc.sync.dma_start(out=cst_sb[:], in_=cst_d[:])
        w2p_sb = cst_sb[:, 0:128]
        sgn_sb = cst_sb[:, 128:129]
        b2p_sb = cst_sb[:, 129:130]
        eps_sb = const.tile([P, 1], F32)
        nc.vector.memset(eps_sb[:], 1e-8)
        zn_sb = const.tile([P, snb, DEG], F32)

        hp_pool = ctx.enter_context(tc.tile_pool(name="hpp", bufs=2))
        h1_pool = ctx.enter_context(tc.tile_pool(name="h1p", bufs=1))
        s_pool = ctx.enter_context(tc.tile_pool(name="sp", bufs=1))
        z_pool = ctx.enter_context(tc.tile_pool(name="zp", bufs=1))
        ps_s = ctx.enter_context(tc.tile_pool(name="pss", bufs=1, space="PSUM"))
        ps_z = ctx.enter_context(tc.tile_pool(name="psz", bufs=1, space="PSUM"))

        rep_ctx = tc.For_i(0, repeat, name="rep")
        _rep = rep_ctx.__enter__()
        blk_ctx = tc.For_i(0, nb, step=2, name="blk")
        bi = blk_ctx.__enter__()
        for u in range(2):
            hp_sb = hp_pool.tile([P, BE], I16, tag=f"hp{u}", name=f"hp{u}")
            nc.sync.dma_start(out=hp_sb[:], in_=hp_d[ts(bi + u, P)])
            h1r = h1_pool.tile([P, BE], F32, tag=f"h1{u}", name=f"h1{u}")
            # dequant + relu in one op
            nc.vector.tensor_scalar(out=h1r[:], in0=hp_sb[:], scalar1=QSCALE,
                                    scalar2=0.0, op0=TT.mult, op1=TT.max)
            z_sb = z_pool.tile([1, BE], F32, tag="z", name="z_sb")
            for g in range(BE // 1024):
                k = g % 2
                pss = ps_s.tile([P, 1024], F32, tag=f"pss{k}", name=f"pss{k}")
                for c in range(2):
                    nc.tensor.matmul(out=pss[:, 512 * c:512 * (c + 1)],
                                     lhsT=w2p_sb,
                                     rhs=h1r[:, 1024 * g + 512 * c:1024 * g + 512 * (c + 1)],
                                     start=True, stop=True)
                s_sb = s_pool.tile([P, 1024], F32, tag=f"s{k}", name=f"s{k}")
                nc.scalar.activation(out=s_sb[:], in_=pss[:],
                                     func=mybir.ActivationFunctionType.Relu,
                                     bias=b2p_sb)
                psz = ps_z.tile([1, 1024], F32, tag=f"psz{k}", name=f"psz{k}")
                for c in range(2):
                    nc.tensor.matmul(out=psz[:, 512 * c:512 * (c + 1)],
                                     lhsT=sgn_sb, rhs=s_sb[:, 512 * c:512 * (c + 1)],
                                     start=True, stop=True)
                zc = z_sb[:, 1024 * g:1024 * (g + 1)]
                if k == 0:
                    nc.vector.tensor_copy(out=zc, in_=psz[:])
                else:
                    nc.scalar.activation(out=zc, in_=psz[:],
                                         func=mybir.ActivationFunctionType.Copy)
            nc.sync.dma_start(out=zscr_d[ts(bi + u, BE)], in_=z_sb[:])
        blk_ctx.__exit__(None, None, None)

        # segment stage: fetch z back node-major [p, b, j] <- z_scratch[b*2048 + p*16 + j]
        znsrc = bass.AP(tensor=zscr_d[:].tensor, offset=zscr_d[:].offset,
                        ap=[[snb * DEG, P], [DEG, snb], [1, DEG]])
        nc.sync.dma_start(out=zn_sb[:], in_=znsrc)
        seg = ctx.enter_context(tc.tile_pool(name="seg", bufs=1))

        def t_new(nm):
            t = seg.tile([P, snb, DEG], F32, tag=nm, name=nm)
            return t

        def bcast(t2):
            a = t2[:]
            return bass.AP(tensor=a.tensor, offset=a.offset,
                           ap=[list(a.ap[0]), list(a.ap[1]), [0, DEG]])

        m1 = seg.tile([P, snb], F32)
        nc.vector.reduce_max(out=m1[:], in_=zn_sb[:], axis=mybir.AxisListType.X)
        e1 = t_new("e1")
        nc.vector.tensor_tensor(out=e1[:], in0=zn_sb[:], in1=bcast(m1),
                                op=TT.subtract)
        nc.scalar.activation(out=e1[:], in_=e1[:],
                             func=mybir.ActivationFunctionType.Exp)
        s1 = seg.tile([P, snb], F32)
        nc.vector.reduce_sum(out=s1[:], in_=e1[:], axis=mybir.AxisListType.X)
        r1 = seg.tile([P, snb], F32)
        nc.vector.reciprocal(out=r1[:], in_=s1[:])
        pi = t_new("pi")
        nc.vector.tensor_tensor(out=pi[:], in0=e1[:], in1=bcast(r1), op=TT.mult)
        hard = t_new("hard")
        nc.scalar.activation(out=hard[:], in_=pi[:],
                             func=mybir.ActivationFunctionType.Ln, bias=eps_sb[:])
        nc.scalar.activation(out=hard[:], in_=hard[:],
                             func=mybir.ActivationFunctionType.Sigmoid,
                             scale=float(inv_temp))
        m2 = seg.tile([P, snb], F32)
        nc.vector.reduce_max(out=m2[:], in_=hard[:], axis=mybir.AxisListType.X)
        e2 = t_new("e2")
        nc.vector.tensor_tensor(out=e2[:], in0=hard[:], in1=bcast(m2),
                                op=TT.subtract)
        nc.scalar.activation(out=e2[:], in_=e2[:],
                             func=mybir.ActivationFunctionType.Exp)
        s2 = seg.tile([P, snb], F32)
        nc.vector.reduce_sum(out=s2[:], in_=e2[:], axis=mybir.AxisListType.X)
        r2 = seg.tile([P, snb], F32)
        nc.vector.reciprocal(out=r2[:], in_=s2[:])
        y = t_new("y")
        nc.vector.tensor_tensor(out=y[:], in0=e2[:], in1=bcast(r2), op=TT.mult)

        A = t_new("A")
        nc.vector.tensor_copy(out=A[:], in_=y[:])
        Tt = seg.tile([P, snb, 8], F32)
        for (p_, d_, r_) in _batcher_substages(16):
            pat = _substage_pattern(16, p_, d_, r_)
            if pat is None:
                continue
            off, ostep, ocnt, icnt = pat

            def sl(extra):
                a = A[:]
                return bass.AP(tensor=a.tensor, offset=a.offset + off + extra,
                               ap=[list(a.ap[0]), [DEG, snb], [ostep, ocnt], [1, icnt]])

            tlo, thi = sl(0), sl(d_)
            tt_ap = bass.AP(tensor=Tt[:].tensor, offset=Tt[:].offset,
                            ap=[list(Tt[:].ap[0]), [8, snb], [icnt, ocnt], [1, icnt]])
            nc.vector.tensor_tensor(out=tt_ap, in0=tlo, in1=thi, op=TT.min)
            nc.vector.tensor_tensor(out=tlo, in0=tlo, in1=thi, op=TT.max)
            nc.vector.tensor_copy(out=thi, in_=tt_ap)
        aA = A[:]
        thre_b = bass.AP(tensor=aA.tensor, offset=aA.offset + 7,
                         ap=[list(aA.ap[0]), [DEG, snb], [0, DEG]])
        g = t_new("g")
        nc.vector.scalar_tensor_tensor(out=g[:], in0=y[:], scalar=1e-7,
                                       in1=thre_b, op0=TT.add, op1=TT.is_gt)
        masked = t_new("masked")
        nc.vector.tensor_tensor(out=masked[:], in0=g[:], in1=y[:], op=TT.mult)

        out_main = bass.AP(tensor=out_d[:].tensor, offset=out_d[:].offset,
                           ap=[[snb * DEG, P - 1], [DEG, snb], [1, DEG]])
        nc.sync.dma_start(out=out_main, in_=masked[0:P - 1, :, :])
        tail_cols = nodes_pc - (P - 1) * snb    # 27
        out_last = bass.AP(tensor=out_d[:].tensor,
                           offset=out_d[:].offset + (P - 1) * snb * DEG,
                           ap=[[DEG, tail_cols], [1, DEG]])
        nc.sync.dma_start(out=out_last, in_=masked[P - 1:P, 0:tail_cols, :])
        rep_ctx.__exit__(None, None, None)
    nc.compile()
    return nc


def _host_prepare(features, indices, values, temperature, w1, b1, w2, b2, w3, b3):
    features = np.asarray(features, np.float32)
    row = np.asarray(indices)[0].astype(np.int64)
    col = np.asarray(indices)[1].astype(np.int64)
    val = np.asarray(values, np.float32).reshape(-1)
    w1 = np.asarray(w1, np.float32)
    u_full = (features @ w1[:D] + np.asarray(b1, np.float32)).astype(np.float32)
    v_full = (features @ w1[D:2 * D]).astype(np.float32)
    w1c = w1[2 * D]
    w3c = np.asarray(w3, np.float32)[:, 0]
    aw3 = np.abs(w3c)
    w2p = np.ascontiguousarray((np.asarray(w2, np.float32) * aw3[None, :]).astype(np.float32))
    sgn = np.sign(w3c).astype(np.float32)
    b2p = (np.asarray(b2, np.float32) * aw3).astype(np.float32)
    cst = np.zeros((P, 130), np.float32)
    cst[:, 0:128] = w2p
    cst[:, 128] = sgn
    cst[:, 129] = b2p
    # layer-1 preactivation per edge, int16-quantized feature-major superblocks
    hpre = u_full[row] + v_full[col] + val[:, None] * w1c[None, :]
    hq_all = np.clip(np.round(hpre * (1.0 / QSCALE)), -32767, 32767).astype(np.int16)
    in_maps = []
    for c in range(N_CORES):
        n0 = c * NODES_PC
        e0 = n0 * DEG
        hpT = np.zeros((NB * P, BE), np.int16)
        for b in range(NB):
            bb = b * BN
            n_nodes = max(0, min(BN, NODES_PC - bb))
            if n_nodes == 0:
                continue
            w = n_nodes * DEG
            sl = slice(e0 + bb * DEG, e0 + bb * DEG + w)
            hpT[b * P:(b + 1) * P, 0:w] = hq_all[sl].T
        in_maps.append({"hpT": hpT, "cst": cst})
    return in_maps


_PROGRAM_CACHE = {}


def kernel(features, indices, values, temperature, w1, b1, w2, b2, w3, b3):
    inv_temp = 1.0 / float(np.asarray(temperature))
    in_maps = _host_prepare(features, indices, values, temperature,
                            w1, b1, w2, b2, w3, b3)
    key = ("v11", inv_temp)
    if key not in _PROGRAM_CACHE:
        _PROGRAM_CACHE[key] = build_program(inv_temp)
    nc = _PROGRAM_CACHE[key]
    trace = bool(os.environ.get("TOPK_TRACE"))
    res = run_bass_kernel_spmd(nc, in_maps, list(range(N_CORES)), trace=trace)
    if trace:
        kernel.last_result = res
    out = np.concatenate([res.results[c]["out"] for c in range(N_CORES)])
    return out.astype(np.float32)
